# revision 32
# baseline (speedup 1.0000x reference)
"""AssignmentLoss kernel for 8 TRN2 NeuronCores.

reference:
    protos = prototypes[labels]                       # [B, D] gather
    cos    = sum(e*p) / (max(||e||,eps)*max(||p||,eps))
    out    = 1 - mean(cos)

Strategy (data-parallel, per the sharding hint):
  - shard embeddings/labels along batch across 8 cores (8192 samples each)
  - replicate the prototype table (bf16, 10MB) in each core's DRAM
  - per core: stream embedding chunks (HWDGE), gather prototype rows with
    indirect DMA (GPSIMD/SWDGE, one row per partition per call), compute
    dot and ||p||^2 with scalar_tensor_tensor+accum on VectorE and ||e||^2
    with Square+accum on ScalarE, small epilogue, PE ones-matmul partition
    reduction -> one scalar per core
  - host sums the 8 scalars (replaces the all-reduce; output is a scalar)

Inputs are cast to bf16 on host: the 2e-2 rel-err budget dwarfs bf16's
~1e-6 contribution to the final mean, and it halves HBM traffic.

Workarounds for this walrus build:
  - any instruction may carry at most ONE sync wait ("Too many sync wait
    commands") -> post-pass hoists extra waits onto same-engine NoOp
    carriers spliced before the instruction
  - tensor_tensor_reduce / custom-DVE ops hit "ISA wrong length" -> use
    scalar_tensor_tensor (accum_out) and activation(Square, accum_out)
"""

import sys

sys.path.insert(0, "/opt/trn_rl_repo")

import ml_dtypes
import numpy as np

from concourse import bass, mybir, tile

B, D, C = 65536, 512, 10000
NCORES = 8
BS = B // NCORES  # 8192 samples per core
P = 128
NT = BS // P  # 64 sample-tiles per core; sample s = p*NT + j
ECH = 8  # embedding tiles per streaming DMA (1MB bf16)

BF16 = mybir.dt.bfloat16
F32 = mybir.dt.float32
I32 = mybir.dt.int32

# Input storage dtype: "bf16" or "fp8" (float8e4m3). Engine op costs are
# FD-driven (identical for both); fp8 halves HBM traffic. rel-err budget
# 2e-2 dwarfs either quantization (~1e-6 bf16 / ~1e-5 fp8 on the output).
IN_DTYPE = "fp8"
_DT_MAP = {
    "bf16": (mybir.dt.bfloat16, ml_dtypes.bfloat16),
    "fp8": (mybir.dt.float8e4, ml_dtypes.float8_e4m3fn),
}

_NC_CACHE = {}


def _split_excess_waits(nc, maxw=1):
    """This walrus build rejects >maxw sync-waits on any instruction.
    Hoist extras onto single-wait NoOp carriers placed just before the
    instruction (engine blocks on each carrier's wait first — an AND of
    waits across consecutive same-engine instructions is equivalent).

    For Tile's kernel-tail Drain (a Drain with many waits, followed by an
    all-engine barrier) the carriers are distributed round-robin across all
    engines: the barrier joins them, so the global wait-set semantics are
    preserved while the chain drains in parallel instead of serially on SP.
    """
    engines = [
        mybir.EngineType.SP,
        mybir.EngineType.Activation,
        mybir.EngineType.DVE,
        mybir.EngineType.PE,
        mybir.EngineType.Pool,
    ]
    n = 0
    for bb in nc.main_func.blocks:
        out = []
        for inst in bb.instructions:
            si = inst.sync_info
            waits = list(si.on_wait) if (si and si.on_wait) else []
            if len(waits) > maxw:
                keep = waits[: maxw]
                extra = waits[maxw:]
                distribute = isinstance(inst, mybir.InstDrain) and len(extra) > 4
                for i, w in enumerate(extra):
                    car = mybir.InstNoOp(name=f"{inst.name}.waitnop{n}")
                    n += 1
                    car.engine = (
                        engines[i % len(engines)] if distribute else inst.engine
                    )
                    car.sync_info = mybir.SyncInfo(on_wait=[w], on_update=[])
                    nc.register_instruction(car, overwrite=True)
                    out.append(car)
                inst.sync_info = mybir.SyncInfo(
                    on_wait=keep, on_update=list(si.on_update or [])
                )
            out.append(inst)
        bb.instructions = out
    return n


def build_nc(repeat=1, p2_dve_num=47, dtype=None):
    """repeat>1 python-unrolls the whole computation — used only by the
    benchmark harness to amortize the ~80ms axon dispatch floor.
    p2_dve_num: how many of the 64 p2 reductions run on VectorE (rest on
    ScalarE); 47 balances the engines under the CoreSim cost model."""
    nc = bass.Bass()
    IND = _DT_MAP[dtype or IN_DTYPE][0]
    emb = nc.declare_dram_parameter("emb", [P, NT * D], IND, False)
    lab = nc.declare_dram_parameter("lab", [P, NT], I32, False)
    tab = nc.declare_dram_parameter("tab", [C, D], IND, False)
    out = nc.declare_dram_parameter("out", [1, 1], F32, True)

    mult = mybir.AluOpType.mult

    with tile.TileContext(nc) as tc:
        with (
            tc.tile_pool(name="io", bufs=3) as io_pool,
            tc.tile_pool(name="gio", bufs=8) as g_pool,
            tc.tile_pool(name="acc", bufs=1) as acc_pool,
            tc.tile_pool(name="scr", bufs=2) as scr_pool,
            tc.tile_pool(name="psum", bufs=1, space="PSUM") as psum_pool,
        ):
            DOT = acc_pool.tile([P, NT], F32, name="DOT")
            E2 = acc_pool.tile([P, NT], F32, name="E2")
            P2 = acc_pool.tile([P, NT], F32, name="P2")
            labs = acc_pool.tile([P, NT], I32, name="labs")
            ones = acc_pool.tile([P, 1], F32, name="ones")
            nc.sync.dma_start(out=labs[:], in_=lab[:])
            nc.vector.memset(ones[:], 1.0)

            # first chunks smaller so DVE/ACT start sooner
            chunk_sizes = [1, 1, 2, 4] + [ECH] * ((NT - 8) // ECH)
            assert sum(chunk_sizes) == NT
            for _rep in range(repeat):
                c0 = 0
                for csz in chunk_sizes:
                    cbase, c0 = c0, c0 + csz
                    et = io_pool.tile([P, ECH * D], IND, tag="emb", name="et")
                    nc.sync.dma_start(
                        out=et[:, : csz * D],
                        in_=emb[:, cbase * D : (cbase + csz) * D],
                    )
                    for j in range(csz):
                        col = cbase + j
                        e_view = et[:, j * D : (j + 1) * D]
                        gt = g_pool.tile([P, D], IND, tag="gath", name="gt")
                        nc.gpsimd.indirect_dma_start(
                            out=gt[:],
                            out_offset=None,
                            in_=tab[:],
                            in_offset=bass.IndirectOffsetOnAxis(
                                ap=labs[:, col : col + 1], axis=0
                            ),
                        )
                        scr = scr_pool.tile([P, D], BF16, tag="scr", name="scr")
                        scr2 = scr_pool.tile([P, D], BF16, tag="scr2", name="scr2")
                        scr3 = scr_pool.tile([P, D], BF16, tag="scr3", name="scr3")
                        # dot = sum(e*g)   (VectorE, fused mul+row-reduce)
                        nc.vector.scalar_tensor_tensor(
                            out=scr[:],
                            in0=e_view,
                            scalar=1.0,
                            in1=gt[:],
                            op0=mult,
                            op1=mult,
                            accum_out=DOT[:, col : col + 1],
                        )
                        # p2 = sum(g*g) — split between VectorE and ScalarE to
                        # balance engine busy (DVE STT=594ns, ACT sq=799ns)
                        if (col * p2_dve_num) % 64 < p2_dve_num:  # interleaved split
                            nc.vector.scalar_tensor_tensor(
                                out=scr2[:],
                                in0=gt[:],
                                scalar=1.0,
                                in1=gt[:],
                                op0=mult,
                                op1=mult,
                                accum_out=P2[:, col : col + 1],
                            )
                        else:
                            nc.scalar.activation(
                                out=scr2[:],
                                in_=gt[:],
                                func=mybir.ActivationFunctionType.Square,
                                accum_out=P2[:, col : col + 1],
                            )
                        # e2 = sum(e*e)    (ScalarE)
                        nc.scalar.activation(
                            out=scr3[:],
                            in_=e_view,
                            func=mybir.ActivationFunctionType.Square,
                            accum_out=E2[:, col : col + 1],
                        )

            # epilogue: cos = dot / sqrt(e2*p2); per-core partial = sum(cos).
            # Split into column halves so the first half overlaps the tail of
            # the main loop (it only needs accumulator columns 0..NT/2).
            den = scr_pool.tile([P, NT], F32, tag="ep0", name="den")
            rec = scr_pool.tile([P, NT], F32, tag="ep1", name="rec")
            cosv = scr_pool.tile([P, NT], F32, tag="ep2", name="cosv")
            srow = scr_pool.tile([P, 2], F32, tag="ep3", name="srow")
            H = NT // 2
            for h in range(2):
                hs = slice(h * H, (h + 1) * H)
                nc.vector.tensor_tensor(
                    out=den[:, hs], in0=E2[:, hs], in1=P2[:, hs], op=mult
                )
                nc.scalar.activation(
                    out=den[:, hs],
                    in_=den[:, hs],
                    func=mybir.ActivationFunctionType.Sqrt,
                )
                nc.vector.reciprocal(out=rec[:, hs], in_=den[:, hs])
                nc.vector.tensor_tensor(
                    out=cosv[:, hs], in0=DOT[:, hs], in1=rec[:, hs], op=mult
                )
                nc.vector.reduce_sum(
                    out=srow[:, h : h + 1], in_=cosv[:, hs], axis=mybir.AxisListType.X
                )
            # partition reduction: out[1,1] = sum over both halves and rows
            srow2 = scr_pool.tile([P, 1], F32, tag="ep5", name="srow2")
            nc.vector.tensor_tensor(
                out=srow2[:], in0=srow[:, 0:1], in1=srow[:, 1:2], op=mybir.AluOpType.add
            )
            ps = psum_pool.tile([1, 1], F32, space="PSUM", name="ps")
            nc.tensor.matmul(
                out=ps[:], lhsT=srow2[:], rhs=ones[:], start=True, stop=True
            )
            res = scr_pool.tile([1, 1], F32, tag="ep4", name="res")
            nc.scalar.copy(out=res[:], in_=ps[:])
            nc.sync.dma_start(out=out[:], in_=res[:])

    _split_excess_waits(nc)
    return nc


def _get_nc():
    if "nc" not in _NC_CACHE:
        _NC_CACHE["nc"] = build_nc()
    return _NC_CACHE["nc"]


def make_in_maps(embeddings, labels, prototypes, dtype=None):
    np_dt = _DT_MAP[dtype or IN_DTYPE][1]
    emb = np.asarray(embeddings, dtype=np.float32).astype(np_dt)
    tab = np.ascontiguousarray(
        np.asarray(prototypes, dtype=np.float32).astype(np_dt)
    )
    lab = np.asarray(labels).astype(np.int32)
    in_maps = []
    for core in range(NCORES):
        esh = np.ascontiguousarray(emb[core * BS : (core + 1) * BS]).reshape(
            P, NT * D
        )
        lsh = np.ascontiguousarray(lab[core * BS : (core + 1) * BS]).reshape(P, NT)
        in_maps.append({"emb": esh, "lab": lsh, "tab": tab})
    return in_maps


def kernel(embeddings, labels, prototypes):
    from concourse.bass_utils import run_bass_kernel_spmd

    nc = _get_nc()
    in_maps = make_in_maps(embeddings, labels, prototypes)
    res = run_bass_kernel_spmd(nc, in_maps, core_ids=list(range(NCORES)))
    total = sum(float(np.asarray(r["out"]).reshape(-1)[0]) for r in res.results)
    return np.float32(1.0 - total / B)


# revision 33
# speedup vs baseline: 1.1699x; 1.1699x over previous
"""AssignmentLoss kernel for 8 TRN2 NeuronCores.

reference:
    protos = prototypes[labels]                       # [B, D] gather
    cos    = sum(e*p) / (max(||e||,eps)*max(||p||,eps))
    out    = 1 - mean(cos)

Strategy (data-parallel, per the sharding hint):
  - shard embeddings/labels along batch across 8 cores (8192 samples each)
  - replicate the prototype table (bf16, 10MB) in each core's DRAM
  - per core: stream embedding chunks in TWO layouts (sample-major for the
    dot, host-pre-transposed D-on-partition for PE), gather prototype rows
    with indirect DMA (GPSIMD/SWDGE, one row per partition per call);
    dot = scalar_tensor_tensor+accum on VectorE, ||p||^2 = Square+accum on
    ScalarE, ||e||^2 = PE pairwise matmul (4 PSUM-accumulated K-chunks) with
    VectorE diagonal extraction vs an identity mask; small epilogue, PE
    ones-matmul partition reduction -> one scalar per core
  - host sums the 8 scalars (replaces the all-reduce; output is a scalar)

Inputs are cast to bf16 on host: the 2e-2 rel-err budget dwarfs bf16's
~1e-6 contribution to the final mean, and it halves HBM traffic.

Workarounds for this walrus build:
  - any instruction may carry at most ONE sync wait ("Too many sync wait
    commands") -> post-pass hoists extra waits onto same-engine NoOp
    carriers spliced before the instruction
  - tensor_tensor_reduce / custom-DVE ops hit "ISA wrong length" -> use
    scalar_tensor_tensor (accum_out) and activation(Square, accum_out)
"""

import sys

sys.path.insert(0, "/opt/trn_rl_repo")

import ml_dtypes
import numpy as np

from concourse import bass, mybir, tile

B, D, C = 65536, 512, 10000
NCORES = 8
BS = B // NCORES  # 8192 samples per core
P = 128
NT = BS // P  # 64 sample-tiles per core; sample s = p*NT + j
ECH = 8  # embedding tiles per streaming DMA (1MB bf16)

BF16 = mybir.dt.bfloat16
F32 = mybir.dt.float32
I32 = mybir.dt.int32

# Input storage dtype: "bf16" or "fp8" (float8e4m3). Engine op costs are
# FD-driven (identical for both); fp8 halves HBM traffic. rel-err budget
# 2e-2 dwarfs either quantization (~1e-6 bf16 / ~1e-5 fp8 on the output).
IN_DTYPE = "fp8"
_DT_MAP = {
    "bf16": (mybir.dt.bfloat16, ml_dtypes.bfloat16),
    "fp8": (mybir.dt.float8e4, ml_dtypes.float8_e4m3fn),
}

_NC_CACHE = {}


def _split_excess_waits(nc, maxw=1):
    """This walrus build rejects >maxw sync-waits on any instruction.
    Hoist extras onto single-wait NoOp carriers placed just before the
    instruction (engine blocks on each carrier's wait first — an AND of
    waits across consecutive same-engine instructions is equivalent).

    For Tile's kernel-tail Drain (a Drain with many waits, followed by an
    all-engine barrier) the carriers are distributed round-robin across all
    engines: the barrier joins them, so the global wait-set semantics are
    preserved while the chain drains in parallel instead of serially on SP.
    """
    engines = [
        mybir.EngineType.SP,
        mybir.EngineType.Activation,
        mybir.EngineType.DVE,
        mybir.EngineType.PE,
        mybir.EngineType.Pool,
    ]
    n = 0
    for bb in nc.main_func.blocks:
        out = []
        for inst in bb.instructions:
            si = inst.sync_info
            waits = list(si.on_wait) if (si and si.on_wait) else []
            if len(waits) > maxw:
                keep = waits[: maxw]
                extra = waits[maxw:]
                distribute = isinstance(inst, mybir.InstDrain) and len(extra) > 4
                for i, w in enumerate(extra):
                    car = mybir.InstNoOp(name=f"{inst.name}.waitnop{n}")
                    n += 1
                    car.engine = (
                        engines[i % len(engines)] if distribute else inst.engine
                    )
                    car.sync_info = mybir.SyncInfo(on_wait=[w], on_update=[])
                    nc.register_instruction(car, overwrite=True)
                    out.append(car)
                inst.sync_info = mybir.SyncInfo(
                    on_wait=keep, on_update=list(si.on_update or [])
                )
            out.append(inst)
        bb.instructions = out
    return n


def build_nc(repeat=1, p2_dve_num=47, dtype=None):
    """repeat>1 python-unrolls the whole computation — used only by the
    benchmark harness to amortize the ~80ms axon dispatch floor.
    p2_dve_num: how many of the 64 p2 reductions run on VectorE (rest on
    ScalarE); 47 balances the engines under the CoreSim cost model."""
    nc = bass.Bass()
    IND = _DT_MAP[dtype or IN_DTYPE][0]
    emb = nc.declare_dram_parameter("emb", [P, NT * D], IND, False)
    embt = nc.declare_dram_parameter("embt", [P, NT * D], IND, False)
    lab = nc.declare_dram_parameter("lab", [P, NT], I32, False)
    tab = nc.declare_dram_parameter("tab", [C, D], IND, False)
    out = nc.declare_dram_parameter("out", [1, 1], F32, True)

    mult = mybir.AluOpType.mult

    with tile.TileContext(nc) as tc:
        with (
            tc.tile_pool(name="io", bufs=3) as io_pool,
            tc.tile_pool(name="gio", bufs=8) as g_pool,
            tc.tile_pool(name="acc", bufs=1) as acc_pool,
            tc.tile_pool(name="scr", bufs=2) as scr_pool,
            tc.tile_pool(name="iot", bufs=3) as iot_pool,
            tc.tile_pool(name="psum", bufs=4, space="PSUM") as psum_pool,
        ):
            DOT = acc_pool.tile([P, NT], F32, name="DOT")
            E2 = acc_pool.tile([P, NT], F32, name="E2")
            P2 = acc_pool.tile([P, NT], F32, name="P2")
            labs = acc_pool.tile([P, NT], I32, name="labs")
            ones = acc_pool.tile([P, 1], F32, name="ones")
            ident = acc_pool.tile([P, P], F32, name="ident")
            nc.sync.dma_start(out=labs[:], in_=lab[:])
            nc.vector.memset(ones[:], 1.0)
            from concourse.masks import make_identity
            make_identity(nc, ident[:])

            # first chunks smaller so DVE/ACT start sooner
            chunk_sizes = [1, 1, 2, 4] + [ECH] * ((NT - 8) // ECH)
            assert sum(chunk_sizes) == NT
            for _rep in range(repeat):
                c0 = 0
                for csz in chunk_sizes:
                    cbase, c0 = c0, c0 + csz
                    et = io_pool.tile([P, ECH * D], IND, tag="emb", name="et")
                    nc.sync.dma_start(
                        out=et[:, : csz * D],
                        in_=emb[:, cbase * D : (cbase + csz) * D],
                    )
                    ett = iot_pool.tile([P, ECH * D], IND, tag="embt", name="ett")
                    nc.sync.dma_start(
                        out=ett[:, : csz * D],
                        in_=embt[:, cbase * D : (cbase + csz) * D],
                    )
                    for j in range(csz):
                        col = cbase + j
                        e_view = et[:, j * D : (j + 1) * D]
                        gt = g_pool.tile([P, D], IND, tag="gath", name="gt")
                        nc.gpsimd.indirect_dma_start(
                            out=gt[:],
                            out_offset=None,
                            in_=tab[:],
                            in_offset=bass.IndirectOffsetOnAxis(
                                ap=labs[:, col : col + 1], axis=0
                            ),
                        )
                        scr = scr_pool.tile([P, D], BF16, tag="scr", name="scr")
                        scr2 = scr_pool.tile([P, D], BF16, tag="scr2", name="scr2")
                        # dot = sum(e*g)   (VectorE, fused mul+row-reduce)
                        nc.vector.scalar_tensor_tensor(
                            out=scr[:],
                            in0=e_view,
                            scalar=1.0,
                            in1=gt[:],
                            op0=mult,
                            op1=mult,
                            accum_out=DOT[:, col : col + 1],
                        )
                        # p2 = sum(g*g)    (ScalarE)
                        nc.scalar.activation(
                            out=scr2[:],
                            in_=gt[:],
                            func=mybir.ActivationFunctionType.Square,
                            accum_out=P2[:, col : col + 1],
                        )
                        # e2 via PE: pairwise dots of the transposed tile
                        # accumulate over 4 K-chunks in PSUM; diagonal holds
                        # ||e||^2; extract with STT vs identity (VectorE)
                        ps = psum_pool.tile([P, P], F32, space="PSUM", name="ps")
                        for cc in range(4):
                            sl = ett[:, j * D + cc * P : j * D + (cc + 1) * P]
                            nc.tensor.matmul(
                                out=ps[:], lhsT=sl, rhs=sl,
                                start=(cc == 0), stop=(cc == 3),
                            )
                        scr4 = scr_pool.tile([P, P], F32, tag="scr4", name="scr4")
                        nc.vector.scalar_tensor_tensor(
                            out=scr4[:],
                            in0=ps[:],
                            scalar=1.0,
                            in1=ident[:],
                            op0=mult,
                            op1=mult,
                            accum_out=E2[:, col : col + 1],
                        )

            # epilogue: cos = dot / sqrt(e2*p2); per-core partial = sum(cos).
            # Split into column halves so the first half overlaps the tail of
            # the main loop (it only needs accumulator columns 0..NT/2).
            den = scr_pool.tile([P, NT], F32, tag="ep0", name="den")
            rec = scr_pool.tile([P, NT], F32, tag="ep1", name="rec")
            cosv = scr_pool.tile([P, NT], F32, tag="ep2", name="cosv")
            srow = scr_pool.tile([P, 2], F32, tag="ep3", name="srow")
            H = NT // 2
            for h in range(2):
                hs = slice(h * H, (h + 1) * H)
                nc.vector.tensor_tensor(
                    out=den[:, hs], in0=E2[:, hs], in1=P2[:, hs], op=mult
                )
                nc.scalar.activation(
                    out=den[:, hs],
                    in_=den[:, hs],
                    func=mybir.ActivationFunctionType.Sqrt,
                )
                nc.vector.reciprocal(out=rec[:, hs], in_=den[:, hs])
                nc.vector.tensor_tensor(
                    out=cosv[:, hs], in0=DOT[:, hs], in1=rec[:, hs], op=mult
                )
                nc.vector.reduce_sum(
                    out=srow[:, h : h + 1], in_=cosv[:, hs], axis=mybir.AxisListType.X
                )
            # partition reduction: out[1,1] = sum over both halves and rows
            srow2 = scr_pool.tile([P, 1], F32, tag="ep5", name="srow2")
            nc.vector.tensor_tensor(
                out=srow2[:], in0=srow[:, 0:1], in1=srow[:, 1:2], op=mybir.AluOpType.add
            )
            ps = psum_pool.tile([1, 1], F32, space="PSUM", name="ps")
            nc.tensor.matmul(
                out=ps[:], lhsT=srow2[:], rhs=ones[:], start=True, stop=True
            )
            res = scr_pool.tile([1, 1], F32, tag="ep4", name="res")
            nc.scalar.copy(out=res[:], in_=ps[:])
            nc.sync.dma_start(out=out[:], in_=res[:])

    _split_excess_waits(nc)
    return nc


def _get_nc():
    if "nc" not in _NC_CACHE:
        _NC_CACHE["nc"] = build_nc()
    return _NC_CACHE["nc"]


def make_in_maps(embeddings, labels, prototypes, dtype=None):
    np_dt = _DT_MAP[dtype or IN_DTYPE][1]
    emb = np.asarray(embeddings, dtype=np.float32).astype(np_dt)
    tab = np.ascontiguousarray(
        np.asarray(prototypes, dtype=np.float32).astype(np_dt)
    )
    lab = np.asarray(labels).astype(np.int32)
    in_maps = []
    for core in range(NCORES):
        shard = emb[core * BS : (core + 1) * BS]
        esh = np.ascontiguousarray(shard).reshape(P, NT * D)
        # transposed layout for PE: [d_low, col*512 + c*128 + p] = e[p*NT+col, c*128+d_low]
        e4 = np.asarray(shard).reshape(P, NT, 4, P)
        tsh = np.ascontiguousarray(e4.transpose(3, 1, 2, 0)).reshape(P, NT * D)
        lsh = np.ascontiguousarray(lab[core * BS : (core + 1) * BS]).reshape(P, NT)
        in_maps.append({"emb": esh, "embt": tsh, "lab": lsh, "tab": tab})
    return in_maps


def kernel(embeddings, labels, prototypes):
    from concourse.bass_utils import run_bass_kernel_spmd

    nc = _get_nc()
    in_maps = make_in_maps(embeddings, labels, prototypes)
    res = run_bass_kernel_spmd(nc, in_maps, core_ids=list(range(NCORES)))
    total = sum(float(np.asarray(r["out"]).reshape(-1)[0]) for r in res.results)
    return np.float32(1.0 - total / B)


# revision 34
# speedup vs baseline: 1.1739x; 1.0034x over previous
"""AssignmentLoss kernel for 8 TRN2 NeuronCores.

reference:
    protos = prototypes[labels]                       # [B, D] gather
    cos    = sum(e*p) / (max(||e||,eps)*max(||p||,eps))
    out    = 1 - mean(cos)

Strategy (data-parallel, per the sharding hint):
  - shard embeddings/labels along batch across 8 cores (8192 samples each)
  - replicate the prototype table (bf16, 10MB) in each core's DRAM
  - per core: stream embedding chunks in TWO layouts (sample-major for the
    dot, host-pre-transposed D-on-partition for PE), gather prototype rows
    with indirect DMA (GPSIMD/SWDGE, one row per partition per call);
    dot = scalar_tensor_tensor+accum on VectorE, ||p||^2 = Square+accum on
    ScalarE, ||e||^2 = PE pairwise matmul (4 PSUM-accumulated K-chunks) with
    VectorE diagonal extraction vs an identity mask; small epilogue, PE
    ones-matmul partition reduction -> one scalar per core
  - host sums the 8 scalars (replaces the all-reduce; output is a scalar)

Inputs are cast to bf16 on host: the 2e-2 rel-err budget dwarfs bf16's
~1e-6 contribution to the final mean, and it halves HBM traffic.

Workarounds for this walrus build:
  - any instruction may carry at most ONE sync wait ("Too many sync wait
    commands") -> post-pass hoists extra waits onto same-engine NoOp
    carriers spliced before the instruction
  - tensor_tensor_reduce / custom-DVE ops hit "ISA wrong length" -> use
    scalar_tensor_tensor (accum_out) and activation(Square, accum_out)
"""

import sys

sys.path.insert(0, "/opt/trn_rl_repo")

import ml_dtypes
import numpy as np

from concourse import bass, mybir, tile

B, D, C = 65536, 512, 10000
NCORES = 8
BS = B // NCORES  # 8192 samples per core
P = 128
NT = BS // P  # 64 sample-tiles per core; sample s = p*NT + j
ECH = 8  # embedding tiles per streaming DMA (1MB bf16)

BF16 = mybir.dt.bfloat16
F32 = mybir.dt.float32
I32 = mybir.dt.int32

# Input storage dtype: "bf16" or "fp8" (float8e4m3). Engine op costs are
# FD-driven (identical for both); fp8 halves HBM traffic. rel-err budget
# 2e-2 dwarfs either quantization (~1e-6 bf16 / ~1e-5 fp8 on the output).
IN_DTYPE = "fp8"
_DT_MAP = {
    "bf16": (mybir.dt.bfloat16, ml_dtypes.bfloat16),
    "fp8": (mybir.dt.float8e4, ml_dtypes.float8_e4m3fn),
}

_NC_CACHE = {}


def _split_excess_waits(nc, maxw=1):
    """This walrus build rejects >maxw sync-waits on any instruction.
    Hoist extras onto single-wait NoOp carriers placed just before the
    instruction (engine blocks on each carrier's wait first — an AND of
    waits across consecutive same-engine instructions is equivalent).

    For Tile's kernel-tail Drain (a Drain with many waits, followed by an
    all-engine barrier) the carriers are distributed round-robin across all
    engines: the barrier joins them, so the global wait-set semantics are
    preserved while the chain drains in parallel instead of serially on SP.
    """
    engines = [
        mybir.EngineType.SP,
        mybir.EngineType.Activation,
        mybir.EngineType.DVE,
        mybir.EngineType.PE,
        mybir.EngineType.Pool,
    ]
    n = 0
    for bb in nc.main_func.blocks:
        out = []
        for inst in bb.instructions:
            si = inst.sync_info
            waits = list(si.on_wait) if (si and si.on_wait) else []
            if len(waits) > maxw:
                keep = waits[: maxw]
                extra = waits[maxw:]
                distribute = isinstance(inst, mybir.InstDrain) and len(extra) > 4
                for i, w in enumerate(extra):
                    car = mybir.InstNoOp(name=f"{inst.name}.waitnop{n}")
                    n += 1
                    car.engine = (
                        engines[i % len(engines)] if distribute else inst.engine
                    )
                    car.sync_info = mybir.SyncInfo(on_wait=[w], on_update=[])
                    nc.register_instruction(car, overwrite=True)
                    out.append(car)
                inst.sync_info = mybir.SyncInfo(
                    on_wait=keep, on_update=list(si.on_update or [])
                )
            out.append(inst)
        bb.instructions = out
    return n


def build_nc(repeat=1, p2_dve_num=47, dtype=None):
    """repeat>1 python-unrolls the whole computation — used only by the
    benchmark harness to amortize the ~80ms axon dispatch floor.
    p2_dve_num: how many of the 64 p2 reductions run on VectorE (rest on
    ScalarE); 47 balances the engines under the CoreSim cost model."""
    nc = bass.Bass()
    IND = _DT_MAP[dtype or IN_DTYPE][0]
    emb = nc.declare_dram_parameter("emb", [P, NT * D], IND, False)
    embt = nc.declare_dram_parameter("embt", [P, NT * D], IND, False)
    lab = nc.declare_dram_parameter("lab", [P, NT], I32, False)
    tab = nc.declare_dram_parameter("tab", [C, D], IND, False)
    out = nc.declare_dram_parameter("out", [1, 1], F32, True)

    mult = mybir.AluOpType.mult

    with tile.TileContext(nc) as tc:
        with (
            tc.tile_pool(name="io", bufs=3) as io_pool,
            tc.tile_pool(name="gio", bufs=8) as g_pool,
            tc.tile_pool(name="acc", bufs=1) as acc_pool,
            tc.tile_pool(name="scr", bufs=2) as scr_pool,
            tc.tile_pool(name="iot", bufs=3) as iot_pool,
            tc.tile_pool(name="psum", bufs=4, space="PSUM") as psum_pool,
        ):
            DOT = acc_pool.tile([P, NT], F32, name="DOT")
            E2 = acc_pool.tile([P, NT], F32, name="E2")
            P2 = acc_pool.tile([P, NT], F32, name="P2")
            labs = acc_pool.tile([P, NT], I32, name="labs")
            ones = acc_pool.tile([P, 1], F32, name="ones")
            ident = acc_pool.tile([P, P], F32, name="ident")
            nc.sync.dma_start(out=labs[:], in_=lab[:])
            nc.vector.memset(ones[:], 1.0)
            from concourse.masks import make_identity
            make_identity(nc, ident[:])

            # first chunks smaller so DVE/ACT start sooner
            chunk_sizes = [1, 1, 2, 4] + [ECH] * ((NT - 8) // ECH)
            assert sum(chunk_sizes) == NT
            for _rep in range(repeat):
                c0 = 0
                for csz in chunk_sizes:
                    cbase, c0 = c0, c0 + csz
                    et = io_pool.tile([P, ECH * D], IND, tag="emb", name="et")
                    nc.sync.dma_start(
                        out=et[:, : csz * D],
                        in_=emb[:, cbase * D : (cbase + csz) * D],
                    )
                    ett = iot_pool.tile([P, ECH * D], IND, tag="embt", name="ett")
                    nc.sync.dma_start(
                        out=ett[:, : csz * D],
                        in_=embt[:, cbase * D : (cbase + csz) * D],
                    )
                    for j in range(csz):
                        col = cbase + j
                        e_view = et[:, j * D : (j + 1) * D]
                        gt = g_pool.tile([P, D], IND, tag="gath", name="gt")
                        nc.gpsimd.indirect_dma_start(
                            out=gt[:],
                            out_offset=None,
                            in_=tab[:],
                            in_offset=bass.IndirectOffsetOnAxis(
                                ap=labs[:, col : col + 1], axis=0
                            ),
                        )
                        scr = scr_pool.tile([P, D], BF16, tag="scr", name="scr")
                        scr2 = scr_pool.tile([P, D], BF16, tag="scr2", name="scr2")
                        # dot = sum(e*g)   (VectorE, fused mul+row-reduce)
                        nc.vector.scalar_tensor_tensor(
                            out=scr[:],
                            in0=e_view,
                            scalar=1.0,
                            in1=gt[:],
                            op0=mult,
                            op1=mult,
                            accum_out=DOT[:, col : col + 1],
                        )
                        # p2 = sum(g*g)    (ScalarE)
                        nc.scalar.activation(
                            out=scr2[:],
                            in_=gt[:],
                            func=mybir.ActivationFunctionType.Square,
                            accum_out=P2[:, col : col + 1],
                        )
                        # e2: 2 tiles on ScalarE to balance DVE/ACT busy
                        if col in (0, 1):
                            scr3 = scr_pool.tile([P, D], BF16, tag="scr3", name="scr3")
                            nc.scalar.activation(
                                out=scr3[:],
                                in_=e_view,
                                func=mybir.ActivationFunctionType.Square,
                                accum_out=E2[:, col : col + 1],
                            )
                            continue
                        # e2 via PE: pairwise dots of the transposed tile
                        # accumulate over 4 K-chunks in PSUM; diagonal holds
                        # ||e||^2; extract with STT vs identity (VectorE)
                        ps = psum_pool.tile([P, P], F32, space="PSUM", name="ps")
                        for cc in range(4):
                            sl = ett[:, j * D + cc * P : j * D + (cc + 1) * P]
                            nc.tensor.matmul(
                                out=ps[:], lhsT=sl, rhs=sl,
                                start=(cc == 0), stop=(cc == 3),
                            )
                        scr4 = scr_pool.tile([P, P], F32, tag="scr4", name="scr4")
                        nc.vector.scalar_tensor_tensor(
                            out=scr4[:],
                            in0=ps[:],
                            scalar=1.0,
                            in1=ident[:],
                            op0=mult,
                            op1=mult,
                            accum_out=E2[:, col : col + 1],
                        )

            # epilogue: cos = dot / sqrt(e2*p2); per-core partial = sum(cos).
            # Split into column halves so the first half overlaps the tail of
            # the main loop (it only needs accumulator columns 0..NT/2).
            den = scr_pool.tile([P, NT], F32, tag="ep0", name="den")
            rec = scr_pool.tile([P, NT], F32, tag="ep1", name="rec")
            cosv = scr_pool.tile([P, NT], F32, tag="ep2", name="cosv")
            srow = scr_pool.tile([P, 2], F32, tag="ep3", name="srow")
            H = NT // 2
            for h in range(2):
                hs = slice(h * H, (h + 1) * H)
                nc.vector.tensor_tensor(
                    out=den[:, hs], in0=E2[:, hs], in1=P2[:, hs], op=mult
                )
                nc.scalar.activation(
                    out=den[:, hs],
                    in_=den[:, hs],
                    func=mybir.ActivationFunctionType.Sqrt,
                )
                nc.vector.reciprocal(out=rec[:, hs], in_=den[:, hs])
                nc.vector.tensor_tensor(
                    out=cosv[:, hs], in0=DOT[:, hs], in1=rec[:, hs], op=mult
                )
                nc.vector.reduce_sum(
                    out=srow[:, h : h + 1], in_=cosv[:, hs], axis=mybir.AxisListType.X
                )
            # partition reduction: out[1,1] = sum over both halves and rows
            srow2 = scr_pool.tile([P, 1], F32, tag="ep5", name="srow2")
            nc.vector.tensor_tensor(
                out=srow2[:], in0=srow[:, 0:1], in1=srow[:, 1:2], op=mybir.AluOpType.add
            )
            ps = psum_pool.tile([1, 1], F32, space="PSUM", name="ps")
            nc.tensor.matmul(
                out=ps[:], lhsT=srow2[:], rhs=ones[:], start=True, stop=True
            )
            res = scr_pool.tile([1, 1], F32, tag="ep4", name="res")
            nc.scalar.copy(out=res[:], in_=ps[:])
            nc.sync.dma_start(out=out[:], in_=res[:])

    _split_excess_waits(nc)
    return nc


def _get_nc():
    if "nc" not in _NC_CACHE:
        _NC_CACHE["nc"] = build_nc()
    return _NC_CACHE["nc"]


def make_in_maps(embeddings, labels, prototypes, dtype=None):
    np_dt = _DT_MAP[dtype or IN_DTYPE][1]
    emb = np.asarray(embeddings, dtype=np.float32).astype(np_dt)
    tab = np.ascontiguousarray(
        np.asarray(prototypes, dtype=np.float32).astype(np_dt)
    )
    lab = np.asarray(labels).astype(np.int32)
    in_maps = []
    for core in range(NCORES):
        shard = emb[core * BS : (core + 1) * BS]
        esh = np.ascontiguousarray(shard).reshape(P, NT * D)
        # transposed layout for PE: [d_low, col*512 + c*128 + p] = e[p*NT+col, c*128+d_low]
        e4 = np.asarray(shard).reshape(P, NT, 4, P)
        tsh = np.ascontiguousarray(e4.transpose(3, 1, 2, 0)).reshape(P, NT * D)
        lsh = np.ascontiguousarray(lab[core * BS : (core + 1) * BS]).reshape(P, NT)
        in_maps.append({"emb": esh, "embt": tsh, "lab": lsh, "tab": tab})
    return in_maps


def kernel(embeddings, labels, prototypes):
    from concourse.bass_utils import run_bass_kernel_spmd

    nc = _get_nc()
    in_maps = make_in_maps(embeddings, labels, prototypes)
    res = run_bass_kernel_spmd(nc, in_maps, core_ids=list(range(NCORES)))
    total = sum(float(np.asarray(r["out"]).reshape(-1)[0]) for r in res.results)
    return np.float32(1.0 - total / B)
